# revision 60
# baseline (speedup 1.0000x reference)
"""Trainium2 Bass kernel for nn_AdaLNConditioning (HGRNBitMLP + AdaLN head).

Strategy (measured 2.11ms -> 1.85ms on HW; PE busy 97.7%, exact math):
- Data-parallel over tokens: 8192 tokens -> 1024 per core, no collectives.
- Host precomputes ternary weight quantization (BitNet b1.58 global-mean
  scale) and packs transposed weight tiles in streaming order as fp8e4
  ({-1,0,1} exact; halves weight HBM traffic vs bf16).
- On device, per token tile [128, D]: RMSNorm stats + per-token int8
  quantization (round-to-nearest-even via the 1.5*2^23 magic constant,
  bit-exact with jnp.round), quantized codes stored as bf16 (integers
  <= 127 are exact in bf16), DMA(XBAR)-transposed into [K, token] layout.
- Matmuls: bf16 stationary (integer codes) x fp8e4 moving (ternary
  weights) with f32 PSUM accumulation -> exact integer arithmetic
  (verified bit-exact on HW); per-token dequant scale applied at PSUM
  evacuation (fused into ScalarE/VectorE copy). PE streaming floor is
  8192 MMs x 216ns = 1.77ms; fp8 DoubleRow cannot beat it exactly
  (int8 codes don't fit e4m3; any exact split doubles the contraction)
  and approximate fp8 quant measures 3-6e-2 e2e rel err (over budget).
- swiglu intermediate z and down-proj output h round-trip through HBM
  in f32 (bf16 storage costs ~1.5e-2 rel err; f32 keeps e2e ~1.4e-3).
- Bottleneck lessons baked in: DMA queues are in-order per issuing
  engine, so weight/read triggers are hoisted a full chunk ahead (and
  never emitted while their buffer is still in use - a WAR-waiting
  trigger head-of-line blocks the whole engine queue); transposes get
  the sync queue to themselves; reads are whole-tile single DMAs
  prefetched one token-tile ahead so the in-order DVE quant chain never
  stalls and delays PSUM evacuation behind it; the startup ramp runs
  chunks 0+1 token-tile-outer with zero-matmul HAM warm-up bursts;
  norm_weight==ones folds the L3 double-RMSNorm into one stats pass
  (general path kept as fallback).
"""

import sys
from contextlib import ExitStack

import numpy as np
import ml_dtypes

sys.path.insert(0, "/opt/trn_rl_repo")

import concourse.bass as bass  # noqa: E402
import concourse.tile as tile  # noqa: E402
from concourse import bacc  # noqa: E402
from concourse import mybir  # noqa: E402
from concourse.masks import make_identity  # noqa: E402

AF = mybir.ActivationFunctionType
ALU = mybir.AluOpType
F32 = mybir.dt.float32
BF16 = mybir.dt.bfloat16

P = 128
MAGIC = 12582912.0  # 1.5 * 2**23: add+store rounds f32 to nearest-even integer
N_CORES = 8


class Cfg:
    def __init__(self, T=1024, D=4096, INTER=4096, CW=512, KB=8):
        self.T = T            # tokens per core
        self.D = D            # model dim (k of L1/L3, out of L2/L3)
        self.INTER = INTER    # swiglu intermediate
        self.CW = CW          # output-chunk width (matmul moving free dim)
        self.KB = KB          # k-tiles per weight DMA batch
        self.TT = T // P
        self.GCH = 2 * INTER // CW  # L1 chunks (v/gate interleaved)
        self.DCH = D // CW          # L2/L3 chunks
        self.KT1 = D // P
        self.KT2 = INTER // P


def host_weight_quant(w):
    """BitNet ternary quant. Returns (codes {-1,0,1} f32, scale) matching
    jnp: scale = 1/clip(mean|w|, 1e-5); q = clip(round(w*scale), -1, 1)."""
    mean_abs = np.mean(np.abs(w), dtype=np.float64).astype(np.float32)
    s = np.float32(1.0) / np.maximum(mean_abs, np.float32(1e-5))
    q = np.clip(np.round(w * s), -1, 1).astype(np.float32)
    return q, s


def pack_weight(WqT, col_starts, cfg):
    """Pack WqT [K, O] into [n_chunks, KG, P, KB, CW] fp8e4 streaming layout.

    Ternary {-1,0,1} codes are exact in fp8e4; the PE multiplies a bf16
    stationary operand against an fp8 moving operand exactly (verified on HW).
    """
    K = WqT.shape[0]
    KT = K // P
    KG = KT // cfg.KB
    out = np.empty((len(col_starts), KG, P, cfg.KB, cfg.CW), dtype=ml_dtypes.float8_e4m3fn)
    for ci, c0 in enumerate(col_starts):
        blk = WqT[:, c0:c0 + cfg.CW]                       # [K, CW]
        blk = blk.reshape(KG, cfg.KB, P, cfg.CW).transpose(0, 2, 1, 3)
        out[ci] = blk.astype(ml_dtypes.float8_e4m3fn)
    return out


def build_nc(cfg, sg, sd, so, nw_ones=False):
    """Build the single-core (SPMD) Bass program."""
    nc = bacc.Bacc()
    T, D, INTER, CW, KB, TT = cfg.T, cfg.D, cfg.INTER, cfg.CW, cfg.KB, cfg.TT
    KT1, KT2, GCH, DCH = cfg.KT1, cfg.KT2, cfg.GCH, cfg.DCH
    KG1, KG2 = KT1 // KB, KT2 // KB
    TH = max(1, TT // 2)          # token tiles per evac half
    NH = (TT + TH - 1) // TH      # evac halves (2)
    QW = min(1024, D)             # quant sub-chunk width

    FP8 = mybir.dt.float8e4
    x_p = nc.declare_dram_parameter("x", [T, D], F32, isOutput=False)
    wg_p = nc.declare_dram_parameter("wg", [GCH, KG1, P, KB, CW], FP8, isOutput=False)
    wd_p = nc.declare_dram_parameter("wd", [DCH, KG2, P, KB, CW], FP8, isOutput=False)
    wo_p = nc.declare_dram_parameter("wo", [DCH, KG1, P, KB, CW], FP8, isOutput=False)
    nw_p = nc.declare_dram_parameter("nw", [1, D], F32, isOutput=False)
    out_p = nc.declare_dram_parameter("out", [T, D], F32, isOutput=True)

    c_gate = float(1.0 / (127.0 * sg))
    c_down = float(1.0 / (127.0 * sd))
    c_out = float(1.0 / (127.0 * so))

    with ExitStack() as ctx:
        tc = ctx.enter_context(tile.TileContext(nc))
        singles = ctx.enter_context(tc.tile_pool(name="singles", bufs=1))
        small = ctx.enter_context(tc.tile_pool(name="small", bufs=48))
        xin = ctx.enter_context(tc.tile_pool(name="xin", bufs=2))      # [P,D] f32
        rts = ctx.enter_context(tc.tile_pool(name="rts", bufs=4))      # [P,QW] f32 scratch
        qt_pool = ctx.enter_context(tc.tile_pool(name="qt", bufs=2))   # [P,KT,TH*P] bf16
        wpool = ctx.enter_context(tc.tile_pool(name="wpool", bufs=4))  # [P,KB,CW] bf16
        gv = ctx.enter_context(tc.tile_pool(name="gv", bufs=2))        # [P,TH,CW] f32 per tag
        zpool = ctx.enter_context(tc.tile_pool(name="zpool", bufs=2))  # [P,TH,CW] f32
        mm_ps = ctx.enter_context(tc.tile_pool(name="mmps", bufs=8, space="PSUM"))
        dram = ctx.enter_context(tc.tile_pool(name="dram", bufs=1, space="DRAM"))

        eps_t = {}
        for ev in (1e-8, 1e-6):
            et = singles.tile([P, 1], F32, name=f"eps{ev}")
            nc.vector.memset(et, ev)
            eps_t[ev] = et
        if not nw_ones:
            nw_bc = singles.tile([P, D], F32)
            nw_ap = nw_p[:]
            nc.sync.dma_start(
                out=nw_bc,
                in_=bass.AP(tensor=nw_ap.tensor, offset=nw_ap.offset,
                            ap=[[0, P], [1, D]]),
            )

        # HAM warm-up fodder: zero matmuls fill PE idle during the startup
        # quant ramp so the clock gate reaches (and keeps) K=8/8.
        dummy_l = singles.tile([P, P], BF16)
        nc.vector.memset(dummy_l, 0.0)
        dummy_r = singles.tile([P, CW], BF16)
        nc.vector.memset(dummy_r, 0.0)
        # allocated first so tag-"mm" rotation stays 4-aligned for real chunks
        dummy_ps = [mm_ps.tile([P, CW], F32, tag="mm", name=f"dps{i}")
                    for i in range(4)]

        def dummy_burst(n):
            for k in range(n):
                nc.tensor.matmul(dummy_ps[k % 4], lhsT=dummy_l, rhs=dummy_r,
                                 start=True, stop=True)

        TPB = min(4, KT1)  # f32 transposes batched per PSUM bank (4*128*4B = 2KB)

        def reduce_cols(parts, fn):
            """Combine [P,1] tiles with a binary DVE op; returns final tile."""
            while len(parts) > 1:
                nxt = []
                for i in range(0, len(parts) - 1, 2):
                    o = small.tile([P, 1], F32, tag="s", name="comb")
                    fn(o, parts[i], parts[i + 1])
                    nxt.append(o)
                if len(parts) % 2:
                    nxt.append(parts[-1])
                parts = nxt
            return parts[0]

        def quant_gen(src_ap, KTn, h, qTt, c_t, c_const, eps, nw=False, nw_eps=None,
                      pre_eps=None):
            """Norm + int8-quant + transpose for token tiles of half h.

            Quant scale is qs = 127/absmax(t2) (the rsqrt factor cancels
            algebraically between quant and dequant); the dequant scale
            c = (am*c_const)*r carries the norm factor r off the critical
            path. rt = (t2*qs + MAGIC) rounds to integer+MAGIC at the f32
            store; the -MAGIC subtract is folded into the PSUM->SBUF
            transpose evacuation (bf16 output, exact for ints <= 127).
            """
            DL = KTn * P
            NQ = DL // QW

            def load_tile(ti, split=False):
                tt_ = h * TH + ti
                x_t = xin.tile([P, DL], F32, tag="xin", bufs=2, name="xt")
                if split:
                    # first tile of a stage: per-sub-chunk DMAs so the stats
                    # chain starts on the first 512KB instead of the full 2MB
                    for j in range(NQ):
                        nc.scalar.dma_start(
                            out=x_t[:, j * QW:(j + 1) * QW],
                            in_=src_ap[tt_ * P:(tt_ + 1) * P, j * QW:(j + 1) * QW])
                else:
                    nc.scalar.dma_start(out=x_t, in_=src_ap[tt_ * P:(tt_ + 1) * P, :])
                return x_t

            pre = {0: load_tile(0, split=True)}
            yield  # reads-only step: tile 0 input en route before any compute
            for i in range(TH):
                tt = h * TH + i
                x_t = pre.pop(i)
                x_js = [x_t[:, j * QW:(j + 1) * QW] for j in range(NQ)]
                # prefetch the whole next tile a full step ahead so the DVE
                # quant chain never stalls at queue head waiting on reads
                if i + 1 < TH:
                    pre[i + 1] = load_tile(i + 1)
                sparts = []
                r = small.tile([P, 1], F32, tag="s")
                aparts = []
                if not nw:
                    # emit abs-max (DVE) interleaved with squares (ACT) per
                    # sub-chunk: the two chains are independent and run on
                    # different engines; maxes must not queue behind the
                    # sumsq reduce (which waits on all squares). Partials go
                    # into packed [P,NQ] tiles so one reduce combines them.
                    am4 = small.tile([P, NQ], F32, tag="s4", bufs=4, name="am4")
                    ss4 = small.tile([P, NQ], F32, tag="s4", bufs=4, name="ss4")
                    for j in range(NQ):
                        nc.vector.tensor_reduce(am4[:, j:j + 1], x_js[j],
                                                axis=mybir.AxisListType.X, op=ALU.max,
                                                apply_absolute_value=True)
                        so_ = rts.tile([P, QW], F32, tag="sq", bufs=1)
                        nc.scalar.activation(so_, x_js[j], AF.Square,
                                             accum_out=ss4[:, j:j + 1])
                    am = small.tile([P, 1], F32, tag="s")
                    nc.vector.tensor_reduce(am, am4, axis=mybir.AxisListType.X,
                                            op=ALU.max)
                    aparts = [am]
                    ssq = small.tile([P, 1], F32, tag="s")
                    nc.vector.tensor_reduce(ssq, ss4, axis=mybir.AxisListType.X,
                                            op=ALU.add)
                    if pre_eps is None:
                        std = small.tile([P, 1], F32, tag="s")
                        nc.scalar.activation(std, ssq, AF.Sqrt, scale=1.0 / DL,
                                             bias=eps_t[eps])
                        nc.vector.reciprocal(r, std)
                    else:
                        # RMSNorm(eps=pre_eps, weight==1) followed by the
                        # bit_linear renorm folds into r = r1*r2 computed from
                        # the single sumsq (codes come from raw rows either way)
                        std1 = small.tile([P, 1], F32, tag="s")
                        nc.scalar.activation(std1, ssq, AF.Sqrt, scale=1.0 / DL,
                                             bias=eps_t[pre_eps])
                        r1 = small.tile([P, 1], F32, tag="s")
                        nc.vector.reciprocal(r1, std1)
                        u = small.tile([P, 1], F32, tag="s")
                        nc.vector.tensor_mul(u, r1, r1)
                        v1 = small.tile([P, 1], F32, tag="s")
                        nc.vector.tensor_mul(v1, u, ssq)
                        std2 = small.tile([P, 1], F32, tag="s")
                        nc.scalar.activation(std2, v1, AF.Sqrt, scale=1.0 / DL,
                                             bias=eps_t[eps])
                        r2 = small.tile([P, 1], F32, tag="s")
                        nc.vector.reciprocal(r2, std2)
                        nc.vector.tensor_mul(r, r1, r2)
                else:
                    # reference: h1 = h * rsqrt(mean h^2 + nw_eps) * nw, then
                    # bit_linear renorms: h2 = h1 * rsqrt(mean h1^2 + eps).
                    # Both fold into one per-token factor r = r1*r2 on (h*nw).
                    for j in range(NQ):
                        so_ = rts.tile([P, QW], F32, tag="sq", bufs=1)
                        ssj = small.tile([P, 1], F32, tag="s", name="ssj")
                        nc.scalar.activation(so_, x_js[j], AF.Square,
                                             accum_out=ssj)
                        sparts.append(ssj)
                    ssq = reduce_cols(sparts, nc.vector.tensor_add)
                    std1 = small.tile([P, 1], F32, tag="s")
                    nc.scalar.activation(std1, ssq, AF.Sqrt, scale=1.0 / DL,
                                         bias=eps_t[nw_eps])
                    r1 = small.tile([P, 1], F32, tag="s")
                    nc.vector.reciprocal(r1, std1)
                    s2parts = []
                    for j in range(NQ):
                        t2j = rts.tile([P, QW], F32, tag="t2", bufs=2)
                        nc.vector.tensor_mul(t2j, x_js[j],
                                             nw_bc[:, j * QW:(j + 1) * QW])
                        so2 = rts.tile([P, QW], F32, tag="sq", bufs=1)
                        ss2j = small.tile([P, 1], F32, tag="s", name="ss2j")
                        nc.scalar.activation(so2, t2j, AF.Square, accum_out=ss2j)
                        s2parts.append(ss2j)
                        amj = small.tile([P, 1], F32, tag="s", name="amj")
                        nc.vector.tensor_reduce(amj, t2j, axis=mybir.AxisListType.X,
                                                op=ALU.max, apply_absolute_value=True)
                        aparts.append(amj)
                    ssq2 = reduce_cols(s2parts, nc.vector.tensor_add)
                    u = small.tile([P, 1], F32, tag="s")
                    nc.vector.tensor_mul(u, r1, r1)
                    w2 = small.tile([P, 1], F32, tag="s")
                    nc.vector.tensor_mul(w2, u, ssq2)
                    std2 = small.tile([P, 1], F32, tag="s")
                    nc.scalar.activation(std2, w2, AF.Sqrt, scale=1.0 / DL, bias=eps_t[eps])
                    r2 = small.tile([P, 1], F32, tag="s")
                    nc.vector.reciprocal(r2, std2)
                    nc.vector.tensor_mul(r, r1, r2)
                am = reduce_cols(aparts, nc.vector.tensor_max)
                invam = small.tile([P, 1], F32, tag="s")
                nc.vector.reciprocal(invam, am)
                qs = small.tile([P, 1], F32, tag="s")
                nc.vector.tensor_scalar_mul(qs, invam, 127.0)
                nc.vector.scalar_tensor_tensor(c_t[:, i:i + 1], am, c_const, r,
                                               op0=ALU.mult, op1=ALU.mult)
                # rt = t2*qs + MAGIC (f32 store rounds to nearest-even int);
                # passes split across DVE/ACT to balance engine load
                rtjs = []
                for j in range(NQ):
                    if not nw:
                        src_j = x_js[j]
                    else:
                        src_j = rts.tile([P, QW], F32, tag="t2", bufs=2)
                        nc.vector.tensor_mul(src_j, x_js[j],
                                             nw_bc[:, j * QW:(j + 1) * QW])
                    rt_j = rts.tile([P, QW], F32, tag="rt", bufs=2 if nw_ones else 1)
                    if j % 2 == 0:
                        nc.vector.tensor_scalar(rt_j, src_j, scalar1=qs, scalar2=MAGIC,
                                                op0=ALU.mult, op1=ALU.add)
                    else:
                        # ACT fma path: same f32 result bits matter only via
                        # the dequant scale; splitting halves DVE latency
                        nc.scalar.activation(rt_j, src_j, AF.Copy, scale=qs,
                                             bias=MAGIC)
                    rtjs.append(rt_j)
                # subtract MAGIC -> bf16 codes, then DMA-transpose into qT.
                # Codes for two sub-chunks share one tile so each transpose
                # trigger (a ~1.3us SP instruction) covers 2*QW columns.
                KTQ = QW // P
                qpair = None
                for j in range(NQ):
                    if j % 2 == 0:
                        qpair = rts.tile([P, 2 * QW], BF16, tag="qb",
                                         bufs=2 if nw_ones else 1, name="qpair")
                        nc.vector.tensor_scalar_add(qpair[:, :QW], rtjs[j], -MAGIC)
                    else:
                        nc.scalar.activation(qpair[:, QW:], rtjs[j], AF.Copy,
                                             bias=-MAGIC)
                        nc.sync.dma_start_transpose(
                            qTt[:, (j - 1) * KTQ:(j + 1) * KTQ, i * P:(i + 1) * P],
                            qpair,
                        )
                yield

        def mm_gen(w_p, nch, KTn, h, qTt, evac, qg=None, ramp_cb=None):
            """Per-chunk matmul emission with one-chunk weight lookahead:
            chunk c+1's DMA triggers are emitted before chunk c's MMs so they
            never queue behind interleaved quant/evac ops on the ACT engine.
            First yield is a preload step (chunk-0 weights en route) that the
            driver advances during the previous stage. If qg is given (first
            stage only), chunk 0 runs token-tile-outer, interleaved with the
            quant generator and HAM warm-up matmuls."""
            def load_w(c):
                # whole chunk in one DMA instruction (trigger instructions on
                # the ACT queue cost ~0.6us each and delay compute ops)
                wt = wpool.tile([P, KTn, CW], FP8, tag="w", name="wt",
                                bufs=3 if nw_ones else 2)
                base = w_p[c]
                src = bass.AP(tensor=base.tensor, offset=base.offset,
                              ap=[[KB * CW, P], [P * KB * CW, KTn // KB],
                                  [CW, KB], [1, CW]])
                nc.scalar.dma_start(out=wt, in_=src)
                return wt

            wts_map = {}

            def ensure_w(c):
                if c < nch and c not in wts_map:
                    wts_map[c] = load_w(c)
                return wts_map.get(c)

            if qg is not None:
                next(qg, None)  # reads-only step: x tile-0 ahead of weights
            ensure_w(0)
            yield
            c0 = 0
            if qg is not None:
                # Startup ramp: chunks 0+1 (a v/gate pair) run token-tile-
                # outer so each quantized tile immediately unlocks ~14us of
                # PE work; swiglu-evac per row. Zero-matmul bursts keep the
                # HAM clock warm while the first tile quantizes.
                rbegin, rrow, rend = ramp_cb
                dummy_burst(135)
                ensure_w(1)
                rbegin()
                pss0 = [mm_ps.tile([P, CW], F32, tag="mm", name=f"rp0{i}")
                        for i in range(TH)]
                pss1 = [mm_ps.tile([P, CW], F32, tag="mm", name=f"rp1{i}")
                        for i in range(TH)]
                for i in range(TH):
                    next(qg, None)
                    for ps, wv in ((pss0[i], wts_map[0]), (pss1[i], wts_map[1])):
                        for kt in range(KTn):
                            nc.tensor.matmul(
                                ps,
                                lhsT=qTt[:, kt, i * P:(i + 1) * P],
                                rhs=wv[:, kt, :],
                                start=(kt == 0),
                                stop=(kt == KTn - 1),
                            )
                    rrow(i, pss0[i], pss1[i])
                rend(h)
                ensure_w(2)
                yield
                yield
                c0 = 2
            for c in range(c0, nch):
                wts = ensure_w(c)
                ensure_w(c + 1)
                if nw_ones:
                    ensure_w(c + 2)  # two-chunk lookahead (bufs=3): ~55us lead
                pss = []
                for i in range(TH):
                    ps = mm_ps.tile([P, CW], F32, tag="mm")
                    pss.append(ps)
                for kt in range(KTn):
                    rhs = wts[:, kt, :]
                    for i in range(TH):
                        nc.tensor.matmul(
                            pss[i],
                            lhsT=qTt[:, kt, i * P:(i + 1) * P],
                            rhs=rhs,
                            start=(kt == 0),
                            stop=(kt == KTn - 1),
                        )
                evac(c, h, pss)
                yield

        # ---- dram intermediates (per half: no false cross-half deps) ----
        z_ds = [dram.tile([T // NH, INTER], F32, name=f"z{h}", tag=f"z{h}")
                for h in range(NH)]
        z_rs = [zd[:].rearrange("(a p) n -> p a n", p=P) for zd in z_ds]
        h_ds = [dram.tile([T // NH, D], F32, name=f"h{h}", tag=f"h{h}")
                for h in range(NH)]
        h_rs = [hd[:].rearrange("(a p) n -> p a n", p=P) for hd in h_ds]
        out_r = out_p[:].rearrange("(a p) n -> p a n", p=P)
        st = {}

        def make_ramp1(c1h):
            """Per-row swiglu evac for the startup ramp (chunks 0+1 of L1h0
            processed token-tile-by-token-tile)."""
            st2 = {}

            def begin():
                st2["v"] = gv.tile([P, TH, CW], F32, tag="v", name="rv")
                st2["sig"] = gv.tile([P, TH, CW], F32, tag="sig", bufs=1, name="rs")
                st2["z"] = zpool.tile([P, TH, CW], F32, tag="z", name="rz")

            def row(i, ps_v, ps_g):
                cc = c1h[:, i:i + 1]
                v_s = st2["v"][:, i, :]
                nc.vector.tensor_scalar(v_s, ps_v, scalar1=cc, scalar2=cc,
                                        op0=ALU.mult, op1=ALU.mult)
                nc.scalar.activation(st2["sig"][:, i, :], ps_g, AF.Sigmoid, scale=cc)
                z_s = st2["z"][:, i, :]
                nc.vector.tensor_mul(z_s, ps_g, st2["sig"][:, i, :])
                nc.vector.tensor_mul(z_s, z_s, v_s)

            def end(h):
                nc.scalar.dma_start(out=z_rs[h][:, :, 0:CW], in_=st2["z"])
            return begin, row, end

        def make_evac1(c1h):
            loc = {}

            def evac1(c, h, pss):
                # chunk order v0,g0,v1,g1,...: v dequantized x c1^2 (extra c1
                # pre-applies gate dequant); z = (psum_g * sigmoid(psum_g*c1)) * v
                gi = c // 2
                if c % 2 == 0:
                    v_t = gv.tile([P, TH, CW], F32, tag="v")
                    for i in range(TH):
                        cc = c1h[:, i:i + 1]
                        nc.vector.tensor_scalar(v_t[:, i, :], pss[i], scalar1=cc,
                                                scalar2=cc, op0=ALU.mult, op1=ALU.mult)
                    loc["v"] = v_t
                else:
                    v_t = loc["v"]
                    sig_t = gv.tile([P, TH, CW], F32, tag="sig", bufs=1)
                    z_t = zpool.tile([P, TH, CW], F32, tag="z")
                    for i in range(TH):
                        cc = c1h[:, i:i + 1]
                        nc.scalar.activation(sig_t[:, i, :], pss[i], AF.Sigmoid, scale=cc)
                        nc.vector.tensor_mul(z_t[:, i, :], pss[i], sig_t[:, i, :])
                    nc.vector.tensor_mul(z_t, z_t, v_t)
                    nc.scalar.dma_start(
                        out=z_rs[h][:, :, gi * CW:(gi + 1) * CW], in_=z_t
                    )
            return evac1

        def make_evac_plain(c_th, dst_r, use_h_offset, last=False):
            def evac(c, h, pss):
                o_t = zpool.tile([P, TH, CW], F32, tag="z")
                row0 = h * TH if use_h_offset else 0
                fin = last and c == DCH - 1
                for i in range(TH):
                    cc = c_th[:, i:i + 1]
                    if i % 2 == 0:
                        nc.vector.tensor_scalar(o_t[:, i, :], pss[i], scalar1=cc,
                                                scalar2=None, op0=ALU.mult)
                    else:
                        nc.scalar.activation(o_t[:, i, :], pss[i], AF.Copy, scale=cc)
                    if fin:
                        # final chunk of the kernel: write per token-tile so
                        # the drain overlaps the remaining evac compute
                        nc.scalar.dma_start(
                            out=dst_r[:, row0 + i:row0 + i + 1, c * CW:(c + 1) * CW],
                            in_=o_t[:, i:i + 1, :],
                        )
                if not fin:
                    nc.scalar.dma_start(
                        out=dst_r[:, row0:row0 + TH, c * CW:(c + 1) * CW], in_=o_t
                    )
            return evac

        # ---- pipelined layers: emission INTERLEAVED so quant(stage k+1)
        # overlaps mm(stage k) on every engine's instruction stream ----
        def stage_factory(L, h):
            def mk():
                ct = singles.tile([P, TH], F32, name=f"c{L}_{h}")
                if L == 1:
                    qT = qt_pool.tile([P, KT1, TH * P], BF16, tag="qt")
                    qg = quant_gen(x_p[:], KT1, h, qT, ct, c_gate, 1e-8)
                    mmf = lambda fq=None: mm_gen(
                        wg_p, GCH, KT1, h, qT, make_evac1(ct), fq,
                        make_ramp1(ct) if fq is not None else None)
                    return qg, mmf, GCH
                if L == 2:
                    qT = qt_pool.tile([P, KT2, TH * P], BF16, tag="qt")
                    qg = quant_gen(z_ds[h][:], KT2, 0, qT, ct, c_down, 1e-8)
                    mmf = lambda fq=None: mm_gen(wd_p, DCH, KT2, h, qT,
                                                 make_evac_plain(ct, h_rs[h], False), fq)
                    return qg, mmf, DCH
                qT = qt_pool.tile([P, KT1, TH * P], BF16, tag="qt")
                if nw_ones:
                    qg = quant_gen(h_ds[h][:], KT1, 0, qT, ct, c_out, 1e-8,
                                   pre_eps=1e-6)
                else:
                    qg = quant_gen(h_ds[h][:], KT1, 0, qT, ct, c_out, 1e-8,
                                   nw=True, nw_eps=1e-6)
                mmf = lambda fq=None: mm_gen(wo_p, DCH, KT1, h, qT,
                                             make_evac_plain(ct, out_r, True,
                                                             last=(h == NH - 1)), fq)
                return qg, mmf, DCH
            return mk

        stage_mks = [stage_factory(L, h) for L in (1, 2, 3) for h in range(NH)]
        qg0, mmf, nch = stage_mks[0]()
        mm = mmf(qg0)
        next(mm)  # stage-0 chunk-0 weight preload
        for k in range(len(stage_mks)):
            if k + 1 < len(stage_mks):
                qn, mmf_n, nch_n = stage_mks[k + 1]()
                mm_next = mmf_n(None)
            else:
                qn = mm_next = None
            ci = 0
            for _ in mm:
                ci += 1
                if qn is not None and ci >= 2:
                    next(qn, None)
                if mm_next is not None and ci == nch - 1:
                    next(mm_next)  # next stage's chunk-0 weight preload
            if qn is not None:
                for _ in qn:
                    pass
            if mm_next is not None:
                mm, nch = mm_next, nch_n

    return nc


def prepare_inputs(condition, w_gate, w_down, norm_weight, w_out, cfg, n_cores=N_CORES):
    """Host-side: quantize+pack weights, shard tokens. Returns (in_maps, scales)."""
    TOK = condition.shape[0] * condition.shape[1]
    X = np.ascontiguousarray(condition.reshape(TOK, cfg.D).astype(np.float32, copy=False))

    Wg, sg = host_weight_quant(np.asarray(w_gate, dtype=np.float32))
    Wd, sd = host_weight_quant(np.asarray(w_down, dtype=np.float32))
    Wo, so = host_weight_quant(np.asarray(w_out, dtype=np.float32))

    # L1 chunk order interleaves v/gate so swiglu can fuse per chunk pair
    l1_cols = []
    for i in range(cfg.INTER // cfg.CW):
        l1_cols += [cfg.INTER + i * cfg.CW, i * cfg.CW]
    WG = pack_weight(Wg.T, l1_cols, cfg)
    WD = pack_weight(Wd.T, [i * cfg.CW for i in range(cfg.D // cfg.CW)], cfg)
    WO = pack_weight(Wo.T, [i * cfg.CW for i in range(cfg.D // cfg.CW)], cfg)

    nw = np.ascontiguousarray(np.asarray(norm_weight, dtype=np.float32).reshape(1, cfg.D))

    in_maps = []
    for i in range(n_cores):
        in_maps.append({
            "x": np.ascontiguousarray(X[i * cfg.T:(i + 1) * cfg.T]),
            "wg": WG, "wd": WD, "wo": WO, "nw": nw,
        })
    return in_maps, (sg, sd, so)


def run(condition, w_gate, w_down, norm_weight, w_out, cfg=None, trace=False, tmpdir=None):
    from concourse.bass_utils import run_bass_kernel_spmd
    if cfg is None:
        cfg = Cfg()
    in_maps, (sg, sd, so) = prepare_inputs(condition, w_gate, w_down, norm_weight, w_out, cfg)
    nw_ones = bool(np.all(np.asarray(norm_weight, dtype=np.float32) == np.float32(1.0)))
    nc = build_nc(cfg, sg, sd, so, nw_ones=nw_ones)
    nc.finalize()
    # transient NRT_EXEC_UNIT_UNRECOVERABLE device crashes recover on retry
    last_err = None
    for attempt in range(3):
        try:
            res = run_bass_kernel_spmd(nc, in_maps, list(range(N_CORES)), trace=trace,
                                       tmpdir=tmpdir)
            break
        except Exception as e:  # noqa: BLE001
            last_err = e
            if attempt == 2:
                raise
            trace = False  # profiler cannot restart after a crashed attempt
            import time as _time
            _time.sleep(20)
    outs = np.concatenate([np.asarray(res.results[i]["out"]) for i in range(N_CORES)], axis=0)
    B, S = condition.shape[0], condition.shape[1]
    Pfull = outs.reshape(B, S, cfg.D)
    H = cfg.D // 2
    return (Pfull[..., :H], Pfull[..., H:]), res


def kernel(condition, w_gate, w_down, norm_weight, w_out):
    (scale, shift), _ = run(condition, w_gate, w_down, norm_weight, w_out)
    return scale, shift



# revision 66
# speedup vs baseline: 1.0011x; 1.0011x over previous
"""Trainium2 Bass kernel for nn_AdaLNConditioning (HGRNBitMLP + AdaLN head).

Strategy (measured 2.11ms -> 1.85ms on HW; PE busy 97.7%, exact math):
- Data-parallel over tokens: 8192 tokens -> 1024 per core, no collectives.
- Host precomputes ternary weight quantization (BitNet b1.58 global-mean
  scale) and packs transposed weight tiles in streaming order as fp8e4
  ({-1,0,1} exact; halves weight HBM traffic vs bf16).
- On device, per token tile [128, D]: RMSNorm stats + per-token int8
  quantization (round-to-nearest-even via the 1.5*2^23 magic constant,
  bit-exact with jnp.round), quantized codes stored as bf16 (integers
  <= 127 are exact in bf16), DMA(XBAR)-transposed into [K, token] layout.
- Matmuls: bf16 stationary (integer codes) x fp8e4 moving (ternary
  weights) with f32 PSUM accumulation -> exact integer arithmetic
  (verified bit-exact on HW); per-token dequant scale applied at PSUM
  evacuation (fused into ScalarE/VectorE copy). PE streaming floor is
  8192 MMs x 216ns = 1.77ms; fp8 DoubleRow cannot beat it exactly
  (int8 codes don't fit e4m3; any exact split doubles the contraction)
  and approximate fp8 quant measures 3-6e-2 e2e rel err (over budget).
- swiglu intermediate z and down-proj output h round-trip through HBM
  in f32 (bf16 storage costs ~1.5e-2 rel err; f32 keeps e2e ~1.4e-3).
- Bottleneck lessons baked in: DMA queues are in-order per issuing
  engine, so weight/read triggers are hoisted a full chunk ahead (and
  never emitted while their buffer is still in use - a WAR-waiting
  trigger head-of-line blocks the whole engine queue); transposes get
  the sync queue to themselves; reads are whole-tile single DMAs
  prefetched one token-tile ahead so the in-order DVE quant chain never
  stalls and delays PSUM evacuation behind it; the startup ramp runs
  chunks 0+1 token-tile-outer with zero-matmul HAM warm-up bursts;
  norm_weight==ones folds the L3 double-RMSNorm into one stats pass
  (general path kept as fallback).
"""

import sys
from contextlib import ExitStack

import numpy as np
import ml_dtypes

sys.path.insert(0, "/opt/trn_rl_repo")

import concourse.bass as bass  # noqa: E402
import concourse.tile as tile  # noqa: E402
from concourse import bacc  # noqa: E402
from concourse import mybir  # noqa: E402
from concourse.masks import make_identity  # noqa: E402

AF = mybir.ActivationFunctionType
ALU = mybir.AluOpType
F32 = mybir.dt.float32
BF16 = mybir.dt.bfloat16

P = 128
MAGIC = 12582912.0  # 1.5 * 2**23: add+store rounds f32 to nearest-even integer
N_CORES = 8


class Cfg:
    def __init__(self, T=1024, D=4096, INTER=4096, CW=512, KB=8):
        self.T = T            # tokens per core
        self.D = D            # model dim (k of L1/L3, out of L2/L3)
        self.INTER = INTER    # swiglu intermediate
        self.CW = CW          # output-chunk width (matmul moving free dim)
        self.KB = KB          # k-tiles per weight DMA batch
        self.TT = T // P
        self.GCH = 2 * INTER // CW  # L1 chunks (v/gate interleaved)
        self.DCH = D // CW          # L2/L3 chunks
        self.KT1 = D // P
        self.KT2 = INTER // P


def host_weight_quant(w):
    """BitNet ternary quant. Returns (codes {-1,0,1} f32, scale) matching
    jnp: scale = 1/clip(mean|w|, 1e-5); q = clip(round(w*scale), -1, 1)."""
    mean_abs = np.mean(np.abs(w), dtype=np.float64).astype(np.float32)
    s = np.float32(1.0) / np.maximum(mean_abs, np.float32(1e-5))
    q = np.clip(np.round(w * s), -1, 1).astype(np.float32)
    return q, s


def pack_weight(WqT, col_starts, cfg):
    """Pack WqT [K, O] into [n_chunks, KG, P, KB, CW] fp8e4 streaming layout.

    Ternary {-1,0,1} codes are exact in fp8e4; the PE multiplies a bf16
    stationary operand against an fp8 moving operand exactly (verified on HW).
    """
    K = WqT.shape[0]
    KT = K // P
    KG = KT // cfg.KB
    out = np.empty((len(col_starts), KG, P, cfg.KB, cfg.CW), dtype=ml_dtypes.float8_e4m3fn)
    for ci, c0 in enumerate(col_starts):
        blk = WqT[:, c0:c0 + cfg.CW]                       # [K, CW]
        blk = blk.reshape(KG, cfg.KB, P, cfg.CW).transpose(0, 2, 1, 3)
        out[ci] = blk.astype(ml_dtypes.float8_e4m3fn)
    return out


def build_nc(cfg, sg, sd, so, nw_ones=False):
    """Build the single-core (SPMD) Bass program."""
    nc = bacc.Bacc()
    T, D, INTER, CW, KB, TT = cfg.T, cfg.D, cfg.INTER, cfg.CW, cfg.KB, cfg.TT
    KT1, KT2, GCH, DCH = cfg.KT1, cfg.KT2, cfg.GCH, cfg.DCH
    KG1, KG2 = KT1 // KB, KT2 // KB
    TH = max(1, TT // 2)          # token tiles per evac half
    NH = (TT + TH - 1) // TH      # evac halves (2)
    QW = min(1024, D)             # quant sub-chunk width

    FP8 = mybir.dt.float8e4
    x_p = nc.declare_dram_parameter("x", [T, D], F32, isOutput=False)
    wg_p = nc.declare_dram_parameter("wg", [GCH, KG1, P, KB, CW], FP8, isOutput=False)
    wd_p = nc.declare_dram_parameter("wd", [DCH, KG2, P, KB, CW], FP8, isOutput=False)
    wo_p = nc.declare_dram_parameter("wo", [DCH, KG1, P, KB, CW], FP8, isOutput=False)
    nw_p = nc.declare_dram_parameter("nw", [1, D], F32, isOutput=False)
    out_p = nc.declare_dram_parameter("out", [T, D], F32, isOutput=True)

    c_gate = float(1.0 / (127.0 * sg))
    c_down = float(1.0 / (127.0 * sd))
    c_out = float(1.0 / (127.0 * so))

    with ExitStack() as ctx:
        tc = ctx.enter_context(tile.TileContext(nc))
        singles = ctx.enter_context(tc.tile_pool(name="singles", bufs=1))
        small = ctx.enter_context(tc.tile_pool(name="small", bufs=48))
        xin = ctx.enter_context(tc.tile_pool(name="xin", bufs=2))      # [P,D] f32
        rts = ctx.enter_context(tc.tile_pool(name="rts", bufs=4))      # [P,QW] f32 scratch
        qt_pool = ctx.enter_context(tc.tile_pool(name="qt", bufs=2))   # [P,KT,TH*P] bf16
        wpool = ctx.enter_context(tc.tile_pool(name="wpool", bufs=4))  # [P,KB,CW] bf16
        gv = ctx.enter_context(tc.tile_pool(name="gv", bufs=2))        # [P,TH,CW] f32 per tag
        zpool = ctx.enter_context(tc.tile_pool(name="zpool", bufs=2))  # [P,TH,CW] f32
        mm_ps = ctx.enter_context(tc.tile_pool(name="mmps", bufs=8, space="PSUM"))
        dram = ctx.enter_context(tc.tile_pool(name="dram", bufs=1, space="DRAM"))

        eps_t = {}
        for ev in (1e-8, 1e-6):
            et = singles.tile([P, 1], F32, name=f"eps{ev}")
            nc.vector.memset(et, ev)
            eps_t[ev] = et
        if not nw_ones:
            nw_bc = singles.tile([P, D], F32)
            nw_ap = nw_p[:]
            nc.sync.dma_start(
                out=nw_bc,
                in_=bass.AP(tensor=nw_ap.tensor, offset=nw_ap.offset,
                            ap=[[0, P], [1, D]]),
            )

        # HAM warm-up fodder: zero matmuls fill PE idle during the startup
        # quant ramp so the clock gate reaches (and keeps) K=8/8.
        dummy_l = singles.tile([P, P], BF16)
        nc.vector.memset(dummy_l, 0.0)
        dummy_r = singles.tile([P, CW], BF16)
        nc.vector.memset(dummy_r, 0.0)
        # allocated first so tag-"mm" rotation stays 4-aligned for real chunks
        dummy_ps = [mm_ps.tile([P, CW], F32, tag="mm", name=f"dps{i}")
                    for i in range(4)]

        def dummy_burst(n):
            for k in range(n):
                nc.tensor.matmul(dummy_ps[k % 4], lhsT=dummy_l, rhs=dummy_r,
                                 start=True, stop=True)

        TPB = min(4, KT1)  # f32 transposes batched per PSUM bank (4*128*4B = 2KB)

        def reduce_cols(parts, fn):
            """Combine [P,1] tiles with a binary DVE op; returns final tile."""
            while len(parts) > 1:
                nxt = []
                for i in range(0, len(parts) - 1, 2):
                    o = small.tile([P, 1], F32, tag="s", name="comb")
                    fn(o, parts[i], parts[i + 1])
                    nxt.append(o)
                if len(parts) % 2:
                    nxt.append(parts[-1])
                parts = nxt
            return parts[0]

        def quant_gen(src_ap, KTn, h, qTt, c_t, c_const, eps, nw=False, nw_eps=None,
                      pre_eps=None):
            """Norm + int8-quant + transpose for token tiles of half h.

            Quant scale is qs = 127/absmax(t2) (the rsqrt factor cancels
            algebraically between quant and dequant); the dequant scale
            c = (am*c_const)*r carries the norm factor r off the critical
            path. rt = (t2*qs + MAGIC) rounds to integer+MAGIC at the f32
            store; the -MAGIC subtract is folded into the PSUM->SBUF
            transpose evacuation (bf16 output, exact for ints <= 127).
            """
            DL = KTn * P
            NQ = DL // QW

            def load_tile(ti, split=False):
                tt_ = h * TH + ti
                x_t = xin.tile([P, DL], F32, tag="xin", bufs=2, name="xt")
                if split:
                    # first tile of a stage: per-sub-chunk DMAs so the stats
                    # chain starts on the first 512KB instead of the full 2MB
                    for j in range(NQ):
                        nc.scalar.dma_start(
                            out=x_t[:, j * QW:(j + 1) * QW],
                            in_=src_ap[tt_ * P:(tt_ + 1) * P, j * QW:(j + 1) * QW])
                else:
                    nc.scalar.dma_start(out=x_t, in_=src_ap[tt_ * P:(tt_ + 1) * P, :])
                return x_t

            pre = {0: load_tile(0, split=True)}
            yield  # reads-only step: tile 0 input en route before any compute
            for i in range(TH):
                tt = h * TH + i
                x_t = pre.pop(i)
                x_js = [x_t[:, j * QW:(j + 1) * QW] for j in range(NQ)]
                # prefetch the whole next tile a full step ahead so the DVE
                # quant chain never stalls at queue head waiting on reads
                if i + 1 < TH:
                    pre[i + 1] = load_tile(i + 1)
                sparts = []
                r = small.tile([P, 1], F32, tag="s")
                aparts = []
                if not nw:
                    # emit abs-max (DVE) interleaved with squares (ACT) per
                    # sub-chunk: the two chains are independent and run on
                    # different engines; maxes must not queue behind the
                    # sumsq reduce (which waits on all squares). Partials go
                    # into packed [P,NQ] tiles so one reduce combines them.
                    am4 = small.tile([P, NQ], F32, tag="s4", bufs=4, name="am4")
                    ss4 = small.tile([P, NQ], F32, tag="s4", bufs=4, name="ss4")
                    for j in range(NQ):
                        nc.vector.tensor_reduce(am4[:, j:j + 1], x_js[j],
                                                axis=mybir.AxisListType.X, op=ALU.max,
                                                apply_absolute_value=True)
                        so_ = rts.tile([P, QW], F32, tag="sq", bufs=1)
                        nc.scalar.activation(so_, x_js[j], AF.Square,
                                             accum_out=ss4[:, j:j + 1])
                    am = small.tile([P, 1], F32, tag="s")
                    nc.vector.tensor_reduce(am, am4, axis=mybir.AxisListType.X,
                                            op=ALU.max)
                    aparts = [am]
                    ssq = small.tile([P, 1], F32, tag="s")
                    nc.vector.tensor_reduce(ssq, ss4, axis=mybir.AxisListType.X,
                                            op=ALU.add)
                    if pre_eps is None:
                        std = small.tile([P, 1], F32, tag="s")
                        nc.scalar.activation(std, ssq, AF.Sqrt, scale=1.0 / DL,
                                             bias=eps_t[eps])
                        nc.vector.reciprocal(r, std)
                    else:
                        # RMSNorm(eps=pre_eps, weight==1) followed by the
                        # bit_linear renorm folds into r = r1*r2 computed from
                        # the single sumsq (codes come from raw rows either way)
                        std1 = small.tile([P, 1], F32, tag="s")
                        nc.scalar.activation(std1, ssq, AF.Sqrt, scale=1.0 / DL,
                                             bias=eps_t[pre_eps])
                        r1 = small.tile([P, 1], F32, tag="s")
                        nc.vector.reciprocal(r1, std1)
                        u = small.tile([P, 1], F32, tag="s")
                        nc.vector.tensor_mul(u, r1, r1)
                        v1 = small.tile([P, 1], F32, tag="s")
                        nc.vector.tensor_mul(v1, u, ssq)
                        std2 = small.tile([P, 1], F32, tag="s")
                        nc.scalar.activation(std2, v1, AF.Sqrt, scale=1.0 / DL,
                                             bias=eps_t[eps])
                        r2 = small.tile([P, 1], F32, tag="s")
                        nc.vector.reciprocal(r2, std2)
                        nc.vector.tensor_mul(r, r1, r2)
                else:
                    # reference: h1 = h * rsqrt(mean h^2 + nw_eps) * nw, then
                    # bit_linear renorms: h2 = h1 * rsqrt(mean h1^2 + eps).
                    # Both fold into one per-token factor r = r1*r2 on (h*nw).
                    for j in range(NQ):
                        so_ = rts.tile([P, QW], F32, tag="sq", bufs=1)
                        ssj = small.tile([P, 1], F32, tag="s", name="ssj")
                        nc.scalar.activation(so_, x_js[j], AF.Square,
                                             accum_out=ssj)
                        sparts.append(ssj)
                    ssq = reduce_cols(sparts, nc.vector.tensor_add)
                    std1 = small.tile([P, 1], F32, tag="s")
                    nc.scalar.activation(std1, ssq, AF.Sqrt, scale=1.0 / DL,
                                         bias=eps_t[nw_eps])
                    r1 = small.tile([P, 1], F32, tag="s")
                    nc.vector.reciprocal(r1, std1)
                    s2parts = []
                    for j in range(NQ):
                        t2j = rts.tile([P, QW], F32, tag="t2", bufs=2)
                        nc.vector.tensor_mul(t2j, x_js[j],
                                             nw_bc[:, j * QW:(j + 1) * QW])
                        so2 = rts.tile([P, QW], F32, tag="sq", bufs=1)
                        ss2j = small.tile([P, 1], F32, tag="s", name="ss2j")
                        nc.scalar.activation(so2, t2j, AF.Square, accum_out=ss2j)
                        s2parts.append(ss2j)
                        amj = small.tile([P, 1], F32, tag="s", name="amj")
                        nc.vector.tensor_reduce(amj, t2j, axis=mybir.AxisListType.X,
                                                op=ALU.max, apply_absolute_value=True)
                        aparts.append(amj)
                    ssq2 = reduce_cols(s2parts, nc.vector.tensor_add)
                    u = small.tile([P, 1], F32, tag="s")
                    nc.vector.tensor_mul(u, r1, r1)
                    w2 = small.tile([P, 1], F32, tag="s")
                    nc.vector.tensor_mul(w2, u, ssq2)
                    std2 = small.tile([P, 1], F32, tag="s")
                    nc.scalar.activation(std2, w2, AF.Sqrt, scale=1.0 / DL, bias=eps_t[eps])
                    r2 = small.tile([P, 1], F32, tag="s")
                    nc.vector.reciprocal(r2, std2)
                    nc.vector.tensor_mul(r, r1, r2)
                am = reduce_cols(aparts, nc.vector.tensor_max)
                invam = small.tile([P, 1], F32, tag="s")
                nc.vector.reciprocal(invam, am)
                qs = small.tile([P, 1], F32, tag="s")
                nc.vector.tensor_scalar_mul(qs, invam, 127.0)
                nc.vector.scalar_tensor_tensor(c_t[:, i:i + 1], am, c_const, r,
                                               op0=ALU.mult, op1=ALU.mult)
                # rt = t2*qs + MAGIC (f32 store rounds to nearest-even int);
                # passes split across DVE/ACT to balance engine load
                rtjs = []
                for j in range(NQ):
                    if not nw:
                        src_j = x_js[j]
                    else:
                        src_j = rts.tile([P, QW], F32, tag="t2", bufs=2)
                        nc.vector.tensor_mul(src_j, x_js[j],
                                             nw_bc[:, j * QW:(j + 1) * QW])
                    rt_j = rts.tile([P, QW], F32, tag="rt", bufs=2 if nw_ones else 1)
                    nc.vector.tensor_scalar(rt_j, src_j, scalar1=qs, scalar2=MAGIC,
                                            op0=ALU.mult, op1=ALU.add)
                    rtjs.append(rt_j)
                # subtract MAGIC -> bf16 codes, then DMA-transpose into qT.
                # Codes for two sub-chunks share one tile so each transpose
                # trigger (a ~1.3us SP instruction) covers 2*QW columns.
                KTQ = QW // P
                qpair = None
                for j in range(NQ):
                    if j % 2 == 0:
                        qpair = rts.tile([P, 2 * QW], BF16, tag="qb",
                                         bufs=2 if nw_ones else 1, name="qpair")
                        nc.vector.tensor_scalar_add(qpair[:, :QW], rtjs[j], -MAGIC)
                    else:
                        nc.scalar.activation(qpair[:, QW:], rtjs[j], AF.Copy,
                                             bias=-MAGIC)
                        nc.sync.dma_start_transpose(
                            qTt[:, (j - 1) * KTQ:(j + 1) * KTQ, i * P:(i + 1) * P],
                            qpair,
                        )
                yield

        def mm_gen(w_p, nch, KTn, h, qTt, evac, qg=None, ramp_cb=None):
            """Per-chunk matmul emission with one-chunk weight lookahead:
            chunk c+1's DMA triggers are emitted before chunk c's MMs so they
            never queue behind interleaved quant/evac ops on the ACT engine.
            First yield is a preload step (chunk-0 weights en route) that the
            driver advances during the previous stage. If qg is given (first
            stage only), chunk 0 runs token-tile-outer, interleaved with the
            quant generator and HAM warm-up matmuls."""
            def load_w(c):
                # whole chunk in one DMA instruction (trigger instructions on
                # the ACT queue cost ~0.6us each and delay compute ops)
                wt = wpool.tile([P, KTn, CW], FP8, tag="w", name="wt",
                                bufs=3 if nw_ones else 2)
                base = w_p[c]
                src = bass.AP(tensor=base.tensor, offset=base.offset,
                              ap=[[KB * CW, P], [P * KB * CW, KTn // KB],
                                  [CW, KB], [1, CW]])
                nc.scalar.dma_start(out=wt, in_=src)
                return wt

            if qg is not None and not nw_ones:
                # fallback path has only 2 weight buffers: no 3-chunk ramp
                for _ in qg:
                    pass
                qg = None
            if qg is not None:
                next(qg, None)  # reads-only step: x tile-0 ahead of weights
            wts_next = load_w(0)
            yield
            c0 = 0
            if qg is not None:
                # Startup ramp: chunks 0+1 (a v/gate pair) run token-tile-
                # outer so each quantized tile immediately unlocks ~14us of
                # PE work; swiglu-evac per row. Zero-matmul bursts keep the
                # HAM clock warm while the first tile quantizes.
                rbegin, rrow, rrow2, rend = ramp_cb
                dummy_burst(135)
                wts1 = load_w(1)
                rbegin()
                pss0 = [mm_ps.tile([P, CW], F32, tag="mm", name=f"rp0{i}")
                        for i in range(TH)]
                pss1 = [mm_ps.tile([P, CW], F32, tag="mm", name=f"rp1{i}")
                        for i in range(TH)]
                wts2 = None
                for i in range(TH):
                    next(qg, None)
                    if i == 0:
                        wts2 = load_w(2)  # after tile-1's read prefetch
                    for ps, wv in ((pss0[i], wts_next), (pss1[i], wts1)):
                        for kt in range(KTn):
                            nc.tensor.matmul(
                                ps,
                                lhsT=qTt[:, kt, i * P:(i + 1) * P],
                                rhs=wv[:, kt, :],
                                start=(kt == 0),
                                stop=(kt == KTn - 1),
                            )
                    rrow(i, pss0[i], pss1[i])
                    # chunk 2 (next pair's v) reuses pss0[i]'s bank: allocated
                    # after rrow's reads so the WAR dependency is tracked
                    ps2 = mm_ps.tile([P, CW], F32, tag="mm", name=f"rp2{i}")
                    for kt in range(KTn):
                        nc.tensor.matmul(
                            ps2,
                            lhsT=qTt[:, kt, i * P:(i + 1) * P],
                            rhs=wts2[:, kt, :],
                            start=(kt == 0),
                            stop=(kt == KTn - 1),
                        )
                    rrow2(i, ps2)
                rend(h)
                wts_next = load_w(3)
                yield
                yield
                yield
                c0 = 3
            for c in range(c0, nch):
                wts = wts_next
                if c + 1 < nch:
                    wts_next = load_w(c + 1)
                pss = []
                for i in range(TH):
                    ps = mm_ps.tile([P, CW], F32, tag="mm")
                    pss.append(ps)
                for kt in range(KTn):
                    rhs = wts[:, kt, :]
                    for i in range(TH):
                        nc.tensor.matmul(
                            pss[i],
                            lhsT=qTt[:, kt, i * P:(i + 1) * P],
                            rhs=rhs,
                            start=(kt == 0),
                            stop=(kt == KTn - 1),
                        )
                evac(c, h, pss)
                yield

        # ---- dram intermediates (per half: no false cross-half deps) ----
        z_ds = [dram.tile([T // NH, INTER], F32, name=f"z{h}", tag=f"z{h}")
                for h in range(NH)]
        z_rs = [zd[:].rearrange("(a p) n -> p a n", p=P) for zd in z_ds]
        h_ds = [dram.tile([T // NH, D], F32, name=f"h{h}", tag=f"h{h}")
                for h in range(NH)]
        h_rs = [hd[:].rearrange("(a p) n -> p a n", p=P) for hd in h_ds]
        out_r = out_p[:].rearrange("(a p) n -> p a n", p=P)
        st = {}

        def make_ramp1(c1h, e1loc):
            """Per-row swiglu evac for the startup ramp (chunks 0..2 of L1h0
            processed token-tile-by-token-tile). Chunk 2 is the next pair's
            v-part; its dequant lands in the steady evac's loc["v"] so the
            post-ramp gate chunk (c=3) finds it."""
            st2 = {}

            def begin():
                st2["v"] = gv.tile([P, TH, CW], F32, tag="v", name="rv")
                st2["sig"] = gv.tile([P, TH, CW], F32, tag="sig", bufs=1, name="rs")
                st2["z"] = zpool.tile([P, TH, CW], F32, tag="z", name="rz")
                e1loc["v"] = gv.tile([P, TH, CW], F32, tag="v", name="rv2")

            def row(i, ps_v, ps_g):
                cc = c1h[:, i:i + 1]
                v_s = st2["v"][:, i, :]
                nc.vector.tensor_scalar(v_s, ps_v, scalar1=cc, scalar2=cc,
                                        op0=ALU.mult, op1=ALU.mult)
                nc.scalar.activation(st2["sig"][:, i, :], ps_g, AF.Sigmoid, scale=cc)
                z_s = st2["z"][:, i, :]
                nc.vector.tensor_mul(z_s, ps_g, st2["sig"][:, i, :])
                nc.vector.tensor_mul(z_s, z_s, v_s)

            def row2(i, ps_v1):
                cc = c1h[:, i:i + 1]
                nc.vector.tensor_scalar(e1loc["v"][:, i, :], ps_v1, scalar1=cc,
                                        scalar2=cc, op0=ALU.mult, op1=ALU.mult)

            def end(h):
                nc.scalar.dma_start(out=z_rs[h][:, :, 0:CW], in_=st2["z"])
            return begin, row, row2, end

        def make_evac1(c1h, loc):

            def evac1(c, h, pss):
                # chunk order v0,g0,v1,g1,...: v dequantized x c1^2 (extra c1
                # pre-applies gate dequant); z = (psum_g * sigmoid(psum_g*c1)) * v
                gi = c // 2
                if c % 2 == 0:
                    v_t = gv.tile([P, TH, CW], F32, tag="v")
                    for i in range(TH):
                        cc = c1h[:, i:i + 1]
                        nc.vector.tensor_scalar(v_t[:, i, :], pss[i], scalar1=cc,
                                                scalar2=cc, op0=ALU.mult, op1=ALU.mult)
                    loc["v"] = v_t
                else:
                    v_t = loc["v"]
                    sig_t = gv.tile([P, TH, CW], F32, tag="sig", bufs=1)
                    z_t = zpool.tile([P, TH, CW], F32, tag="z")
                    for i in range(TH):
                        cc = c1h[:, i:i + 1]
                        nc.scalar.activation(sig_t[:, i, :], pss[i], AF.Sigmoid, scale=cc)
                        nc.vector.tensor_mul(z_t[:, i, :], pss[i], sig_t[:, i, :])
                    nc.vector.tensor_mul(z_t, z_t, v_t)
                    nc.scalar.dma_start(
                        out=z_rs[h][:, :, gi * CW:(gi + 1) * CW], in_=z_t
                    )
            return evac1

        def make_evac_plain(c_th, dst_r, use_h_offset, last=False):
            def evac(c, h, pss):
                o_t = zpool.tile([P, TH, CW], F32, tag="z")
                row0 = h * TH if use_h_offset else 0
                fin = last and c == DCH - 1
                for i in range(TH):
                    cc = c_th[:, i:i + 1]
                    if i % 2 == 0:
                        nc.vector.tensor_scalar(o_t[:, i, :], pss[i], scalar1=cc,
                                                scalar2=None, op0=ALU.mult)
                    else:
                        nc.scalar.activation(o_t[:, i, :], pss[i], AF.Copy, scale=cc)
                    if fin:
                        # final chunk of the kernel: write per token-tile so
                        # the drain overlaps the remaining evac compute
                        nc.scalar.dma_start(
                            out=dst_r[:, row0 + i:row0 + i + 1, c * CW:(c + 1) * CW],
                            in_=o_t[:, i:i + 1, :],
                        )
                if not fin:
                    nc.scalar.dma_start(
                        out=dst_r[:, row0:row0 + TH, c * CW:(c + 1) * CW], in_=o_t
                    )
            return evac

        # ---- pipelined layers: emission INTERLEAVED so quant(stage k+1)
        # overlaps mm(stage k) on every engine's instruction stream ----
        def stage_factory(L, h):
            def mk():
                ct = singles.tile([P, TH], F32, name=f"c{L}_{h}")
                if L == 1:
                    qT = qt_pool.tile([P, KT1, TH * P], BF16, tag="qt")
                    qg = quant_gen(x_p[:], KT1, h, qT, ct, c_gate, 1e-8)
                    e1loc = {}
                    mmf = lambda fq=None: mm_gen(
                        wg_p, GCH, KT1, h, qT, make_evac1(ct, e1loc), fq,
                        make_ramp1(ct, e1loc) if fq is not None else None)
                    return qg, mmf, GCH
                if L == 2:
                    qT = qt_pool.tile([P, KT2, TH * P], BF16, tag="qt")
                    qg = quant_gen(z_ds[h][:], KT2, 0, qT, ct, c_down, 1e-8)
                    mmf = lambda fq=None: mm_gen(wd_p, DCH, KT2, h, qT,
                                                 make_evac_plain(ct, h_rs[h], False), fq)
                    return qg, mmf, DCH
                qT = qt_pool.tile([P, KT1, TH * P], BF16, tag="qt")
                if nw_ones:
                    qg = quant_gen(h_ds[h][:], KT1, 0, qT, ct, c_out, 1e-8,
                                   pre_eps=1e-6)
                else:
                    qg = quant_gen(h_ds[h][:], KT1, 0, qT, ct, c_out, 1e-8,
                                   nw=True, nw_eps=1e-6)
                mmf = lambda fq=None: mm_gen(wo_p, DCH, KT1, h, qT,
                                             make_evac_plain(ct, out_r, True,
                                                             last=(h == NH - 1)), fq)
                return qg, mmf, DCH
            return mk

        stage_mks = [stage_factory(L, h) for L in (1, 2, 3) for h in range(NH)]
        qg0, mmf, nch = stage_mks[0]()
        mm = mmf(qg0)
        next(mm)  # stage-0 chunk-0 weight preload
        for k in range(len(stage_mks)):
            if k + 1 < len(stage_mks):
                qn, mmf_n, nch_n = stage_mks[k + 1]()
                mm_next = mmf_n(None)
            else:
                qn = mm_next = None
            ci = 0
            for _ in mm:
                ci += 1
                if qn is not None and ci >= 2:
                    next(qn, None)
                if mm_next is not None and ci == nch - 1:
                    next(mm_next)  # next stage's chunk-0 weight preload
            if qn is not None:
                for _ in qn:
                    pass
            if mm_next is not None:
                mm, nch = mm_next, nch_n

    return nc


def prepare_inputs(condition, w_gate, w_down, norm_weight, w_out, cfg, n_cores=N_CORES):
    """Host-side: quantize+pack weights, shard tokens. Returns (in_maps, scales)."""
    TOK = condition.shape[0] * condition.shape[1]
    X = np.ascontiguousarray(condition.reshape(TOK, cfg.D).astype(np.float32, copy=False))

    Wg, sg = host_weight_quant(np.asarray(w_gate, dtype=np.float32))
    Wd, sd = host_weight_quant(np.asarray(w_down, dtype=np.float32))
    Wo, so = host_weight_quant(np.asarray(w_out, dtype=np.float32))

    # L1 chunk order interleaves v/gate so swiglu can fuse per chunk pair
    l1_cols = []
    for i in range(cfg.INTER // cfg.CW):
        l1_cols += [cfg.INTER + i * cfg.CW, i * cfg.CW]
    WG = pack_weight(Wg.T, l1_cols, cfg)
    WD = pack_weight(Wd.T, [i * cfg.CW for i in range(cfg.D // cfg.CW)], cfg)
    WO = pack_weight(Wo.T, [i * cfg.CW for i in range(cfg.D // cfg.CW)], cfg)

    nw = np.ascontiguousarray(np.asarray(norm_weight, dtype=np.float32).reshape(1, cfg.D))

    in_maps = []
    for i in range(n_cores):
        in_maps.append({
            "x": np.ascontiguousarray(X[i * cfg.T:(i + 1) * cfg.T]),
            "wg": WG, "wd": WD, "wo": WO, "nw": nw,
        })
    return in_maps, (sg, sd, so)


def run(condition, w_gate, w_down, norm_weight, w_out, cfg=None, trace=False, tmpdir=None):
    from concourse.bass_utils import run_bass_kernel_spmd
    if cfg is None:
        cfg = Cfg()
    in_maps, (sg, sd, so) = prepare_inputs(condition, w_gate, w_down, norm_weight, w_out, cfg)
    nw_ones = bool(np.all(np.asarray(norm_weight, dtype=np.float32) == np.float32(1.0)))
    nc = build_nc(cfg, sg, sd, so, nw_ones=nw_ones)
    nc.finalize()
    # transient NRT_EXEC_UNIT_UNRECOVERABLE device crashes recover on retry
    last_err = None
    for attempt in range(3):
        try:
            res = run_bass_kernel_spmd(nc, in_maps, list(range(N_CORES)), trace=trace,
                                       tmpdir=tmpdir)
            break
        except Exception as e:  # noqa: BLE001
            last_err = e
            if attempt == 2:
                raise
            trace = False  # profiler cannot restart after a crashed attempt
            import time as _time
            _time.sleep(20)
    outs = np.concatenate([np.asarray(res.results[i]["out"]) for i in range(N_CORES)], axis=0)
    B, S = condition.shape[0], condition.shape[1]
    Pfull = outs.reshape(B, S, cfg.D)
    H = cfg.D // 2
    return (Pfull[..., :H], Pfull[..., H:]), res


def kernel(condition, w_gate, w_down, norm_weight, w_out):
    (scale, shift), _ = run(condition, w_gate, w_down, norm_weight, w_out)
    return scale, shift

